# revision 1
# baseline (speedup 1.0000x reference)
"""Trainium2 Bass kernel for nn_AttentionOrig (B=128, C=49, N=3136).

Key algebraic factorization of the reference:
  - attn-after-maxpool is (B,C,1) and v is (B,1,N), so attn@v is an outer
    product and the final 1x1 conv factorizes:
        out[b, m, c] = attnmax[b, c] * (Wproj @ v[b])[m]
  - k = Wsr*x + bsr folds into the score matmul:
        attn[b] = (q[b] @ Wsr1) @ x[b] + (q[b] @ bsr) 1^T
  - the stride-8 q-conv is a sum of 64 per-offset (49x49) matmuls over a
    host-side (kh,kw)-permuted x layout; the same permuted layout feeds the
    score matmul and v (max over n and the Wproj contraction are
    permutation-invariant once Wproj rows are permuted to match).

Two SPMD launches over 8 cores:
  phase 1 (batch-parallel, 16 batches/core): q, G=q@Wsr1, attn rows + fused
    channel-sum row, running max -> per-core v-sums (16,3136), attnmax (16,49).
  phase 2 (Wproj-row-parallel, 392 rows/core): pv = Wproj_shard @ v for all
    128 batches + outer product with attnmax -> out[:, m_shard, :].

Written in raw Bass (manual semaphores): the walrus build here allows only
ONE semaphore wait per instruction, which Tile's auto-generated waits
violate; explicit standalone wait_ge instructions sidestep that.
"""
import sys
from contextlib import ExitStack

import numpy as np

sys.path.insert(0, "/opt/trn_rl_repo")

import concourse.bass as bass
import concourse.mybir as mybir
from concourse.bass_utils import run_bass_kernel_spmd

F32 = mybir.dt.float32
F32R = mybir.dt.float32r
BF16 = mybir.dt.bfloat16

B, C, N = 128, 49, 3136
NCORES = 8
B_LOC = B // NCORES          # 16 batches per core in phase 1
HB = 8                       # batches per partition-half
CH = 392                     # free-dim chunk (one PSUM bank, f32r fast path)
NCH = N // CH                # 8 chunks per batch row
G64 = 64                     # 8x8 conv offsets
MSH = N // NCORES            # 392 Wproj rows per core in phase 2
MB = 49                      # m-block of the outer product
NMB = MSH // MB              # 8 output blocks per core
KP = 112                     # phase-2 contraction chunk: 3136 = 28*112
NKC = N // KP
XP = 2 * CH                  # x-load piece (784 cols)
NXP = N // XP                # 4 pieces per half
NEG_INF = -3.0e38
NAT = 3                      # attn PSUM banks in rotation


def round_f32r(a: np.ndarray) -> np.ndarray:
    """Round-to-nearest-even fp32 -> fp32r (1s/8e/11m, low 12 bits zero)."""
    u = np.ascontiguousarray(a, dtype=np.float32).view(np.uint32)
    u = u + 0x7FF + ((u >> 12) & 1)
    u = u & np.uint32(0xFFFFF000)
    return u.view(np.float32)


# --------------------------------------------------------- phase 1 ledger ---
# DVE tick after each instruction (then_inc(s_dve, 1) on every DVE op):
#   memsets (ones, gtv 1.0, gtv 0, qb_t)  -> 1..4
#   q copies A/B                          -> 5, 6
#   qb stts (16)                          -> 7..22
#   qT copies b=0..15                     -> 23+b  (A half b<8, B half b>=8)
#   gtv copies A/B                        -> 39, 40
#   TTR i=0..63                           -> 41+i
#   final max reduce, am add              -> 105, 106
T_QCOPY = 6
T_QT = lambda b: 23 + b
T_QTB_END = 38
T_GTV = 40
T_TTR = lambda i: 41 + i
T_FIN = 106
# PE ticks (sparse: inc only where a consumer waits):
#   last q mm -> 1; transpose b -> 2+b; mg1 -> 18; mg2 -> 19; attn pair i
#   (inc on mmB) -> 20+i
P_QMM_END = 1
P_TP = lambda b: 2 + b
P_MG1 = 18
P_MG2 = 19
P_AT = lambda i: 20 + i


def build_phase1() -> bass.Bass:
    nc = bass.Bass()
    xs = nc.declare_dram_parameter("xs", [2 * NXP, C, HB * XP], BF16, isOutput=False)
    wq = nc.declare_dram_parameter("wq", [C, G64 * C], BF16, isOutput=False)
    wsr = nc.declare_dram_parameter("wsr", [128, C], BF16, isOutput=False)
    bsr = nc.declare_dram_parameter("bsr", [128, C], F32, isOutput=False)
    ident = nc.declare_dram_parameter("ident", [128, C], F32, isOutput=False)
    v_out = nc.declare_dram_parameter("v_out", [B_LOC, N], F32, isOutput=True)
    am_out = nc.declare_dram_parameter("am_out", [128, HB], F32, isOutput=True)

    with ExitStack() as ctx:
        ec = ctx.enter_context
        # SBUF
        wqt = ec(nc.sbuf_tensor("wqt", [128, G64 * C], BF16))
        wsrt = ec(nc.sbuf_tensor("wsrt", [128, C], BF16))
        identt = ec(nc.sbuf_tensor("identt", [128, C], F32))
        bsrt = ec(nc.sbuf_tensor("bsrt", [128, C], F32))
        ones_sb = ec(nc.sbuf_tensor("ones_sb", [128, CH], F32))
        xall = ec(nc.sbuf_tensor("xall", [128, HB * N], BF16))
        q_sb = ec(nc.sbuf_tensor("q_sb", [128, CH], F32))
        qb_scr = ec(nc.sbuf_tensor("qb_scr", [128, B_LOC * C], F32))
        qb_t = ec(nc.sbuf_tensor("qb_t", [128, HB], F32))
        qT_all = ec(nc.sbuf_tensor("qT_all", [128, CH], BF16))
        qTB = ec(nc.sbuf_tensor("qTB", [128, CH], BF16))
        gtv = ec(nc.sbuf_tensor("gtv", [128, HB * 64], BF16))
        maxacc = ec(nc.sbuf_tensor("maxacc", [128, HB * NCH], F32))
        am1 = ec(nc.sbuf_tensor("am1", [128, HB], F32))
        am_f = ec(nc.sbuf_tensor("am_f", [128, HB], F32))
        vtmp = [ec(nc.sbuf_tensor(f"vtmp{i}", [128, N], F32)) for i in range(2)]
        # PSUM: qpsA qpsB tp0 tp1 gt at0..at2 = 8 banks
        qpsA = ec(nc.psum_tensor("qpsA", [128, CH], F32))
        qpsB = ec(nc.psum_tensor("qpsB", [128, CH], F32))
        tp = [ec(nc.psum_tensor(f"tp{i}", [128, C], F32)) for i in range(2)]
        gtps = ec(nc.psum_tensor("gtps", [128, CH], F32))
        at = [ec(nc.psum_tensor(f"at{i}", [128, CH], F32)) for i in range(NAT)]

        s_w = ec(nc.semaphore("s_w"))
        s_x = [ec(nc.semaphore(f"s_x{i}")) for i in range(NXP)]
        s_pe = ec(nc.semaphore("s_pe"))
        s_dve = ec(nc.semaphore("s_dve"))
        s_qtb = ec(nc.semaphore("s_qtb"))
        s_act = ec(nc.semaphore("s_act"))
        s_vdma = [ec(nc.semaphore(f"s_vd{i}")) for i in range(2)]
        s_am = ec(nc.semaphore("s_am"))

        gtv_r = gtv[:].rearrange("p (b t) -> p b t", b=HB)
        gt_r = gtps[:].rearrange("p (b t) -> p b t", b=HB)

        with nc.Block() as block:

            @block.sync
            def _(sp):
                def xpiece(j, h):
                    rb = 64 * h
                    out_r = xall[rb:rb + C, :].rearrange("p (b n) -> p b n", b=HB)
                    sp.dma_start(
                        out=out_r[:, :, j * XP:(j + 1) * XP],
                        in_=xs[h * NXP + j, :, :].rearrange("c (b n) -> c b n", b=HB),
                    ).then_inc(s_x[j], 16)

                # wq + the first x piece-pair first: they gate the PE start
                sp.dma_start(out=wqt[0:C, :], in_=wq[:]).then_inc(s_w, 16)
                sp.dma_start(out=wqt[64:64 + C, :], in_=wq[:]).then_inc(s_w, 16)
                xpiece(0, 0)
                xpiece(0, 1)
                sp.dma_start(out=wsrt[:], in_=wsr[:]).then_inc(s_w, 16)
                sp.dma_start(out=identt[:], in_=ident[:]).then_inc(s_w, 16)
                sp.dma_start(out=bsrt[:], in_=bsr[:]).then_inc(s_w, 16)
                for j in range(1, NXP):
                    for h in range(2):
                        xpiece(j, h)
                # qTB partition shift once the B-half qT copies are done
                sp.wait_ge(s_dve, T_QTB_END)
                sp.dma_start(out=qT_all[64:64 + C, :], in_=qTB[0:C, :]).then_inc(s_qtb, 16)
                # v rows per pass
                for pa in range(HB):
                    sp.wait_ge(s_act, NCH * (pa + 1))
                    sp.dma_start(out=v_out[pa:pa + 1, :],
                                 in_=vtmp[pa % 2][49:50, :]).then_inc(s_vdma[pa % 2], 16)
                    sp.dma_start(out=v_out[pa + HB:pa + HB + 1, :],
                                 in_=vtmp[pa % 2][113:114, :]).then_inc(s_vdma[pa % 2], 16)
                # attnmax: natural (co-major) layout; host de-interleaves
                sp.wait_ge(s_dve, T_FIN)
                sp.dma_start(out=am_out[:], in_=am_f[:]).then_inc(s_am, 16)
                # retire only when outputs landed
                sp.wait_ge(s_vdma[0], 32 * HB // 2)
                sp.wait_ge(s_vdma[1], 32 * HB // 2)
                sp.wait_ge(s_am, 16)

            @block.tensor
            def _(pe):
                p = 0

                def inc(inst):
                    nonlocal p
                    p += 1
                    inst.then_inc(s_pe, 1)

                pe.wait_ge(s_w, 5 * 16)
                xA = xall[0:C, :].rearrange("p (b n) -> p b n", b=HB)
                xB = xall[64:64 + C, :].rearrange("p (b n) -> p b n", b=HB)
                for g in range(G64):
                    if (g * C) % XP == 0:
                        k = (g * C) // XP
                        pe.wait_ge(s_x[k], 32)
                    nc.tensor.matmul(
                        qpsA[0:C, :],
                        wqt[0:C, g * C:(g + 1) * C],
                        xA[:, :, g * C:(g + 1) * C],
                        start=(g == 0), stop=(g == G64 - 1),
                    )
                    mmb = nc.tensor.matmul(
                        qpsB[64:64 + C, :],
                        wqt[64:64 + C, g * C:(g + 1) * C],
                        xB[:, :, g * C:(g + 1) * C],
                        start=(g == 0), stop=(g == G64 - 1),
                        tile_position=(64, 64),
                    )
                    if g == G64 - 1:
                        inc(mmb)
                assert p == P_QMM_END
                # transposes (tp banks ping-pong)
                for b in range(B_LOC):
                    rb = 0 if b < HB else 64
                    bl = b % HB
                    if b < 2:
                        pe.wait_ge(s_dve, T_QCOPY)
                    else:
                        pe.wait_ge(s_dve, T_QT(b - 2))
                    kw = {} if rb == 0 else {"tile_position": (64, 0)}
                    inc(nc.tensor.transpose(
                        tp[b % 2][0:C, 0:C],
                        q_sb[rb:rb + C, bl * C:(bl + 1) * C],
                        identt[rb:rb + C, :],
                        **kw,
                    ))
                    assert p == P_TP(b)
                # GT
                pe.wait_ge(s_dve, T_QT(HB - 1))
                inc(nc.tensor.matmul(gtps[0:C, :], wsrt[0:C, :], qT_all[0:C, :],
                                     start=True, stop=True))
                assert p == P_MG1
                pe.wait_ge(s_qtb, 16)
                inc(nc.tensor.matmul(gtps[64:64 + C, :], wsrt[64:64 + C, :],
                                     qT_all[64:64 + C, :],
                                     start=True, stop=True, tile_position=(64, 64)))
                assert p == P_MG2
                # attn
                for i in range(HB * NCH):
                    pa, chk = divmod(i, NCH)
                    if i < NAT:
                        pe.wait_ge(s_dve, T_GTV)
                    else:
                        pe.wait_ge(s_dve, T_TTR(i - NAT))
                        pe.wait_ge(s_act, i - NAT + 1)
                    bank = at[i % NAT]
                    nc.tensor.matmul(
                        bank[0:64, :],
                        gtv[0:C, pa * 64:(pa + 1) * 64],
                        xall[0:C, pa * N + chk * CH:pa * N + (chk + 1) * CH],
                        start=True, stop=True,
                    )
                    inc(nc.tensor.matmul(
                        bank[64:128, :],
                        gtv[64:64 + C, pa * 64:(pa + 1) * 64],
                        xall[64:64 + C, pa * N + chk * CH:pa * N + (chk + 1) * CH],
                        start=True, stop=True, tile_position=(64, 64),
                    ))
                    assert p == P_AT(i)

            @block.scalar
            def _(act):
                a_t = 0
                for i in range(HB * NCH):
                    pa, chk = divmod(i, NCH)
                    if chk == 0 and pa >= 2:
                        act.wait_ge(s_vdma[pa % 2], 32 * ((pa - 2) // 2 + 1))
                        act.wait_ge(s_act, NCH * (pa - 1))
                    act.wait_ge(s_pe, P_AT(i))
                    # DVE and ACT must not read the same PSUM bank in
                    # parallel: run the copy strictly after the max-reduce
                    act.wait_ge(s_dve, T_TTR(i))
                    nc.scalar.copy(
                        out=vtmp[pa % 2][:, chk * CH:(chk + 1) * CH],
                        in_=at[i % NAT][:],
                    ).then_inc(s_act, 1)
                    a_t += 1
                assert a_t == HB * NCH

            @block.vector
            def _(dve):
                t = 0

                def inc(inst):
                    nonlocal t
                    t += 1
                    inst.then_inc(s_dve, 1)

                inc(nc.vector.memset(ones_sb[:], 1.0))
                inc(nc.vector.memset(gtv_r[:, :, C:C + 1], 1.0))
                inc(nc.vector.memset(gtv_r[:, :, C + 1:64], 0.0))
                inc(nc.vector.memset(qb_t[:], 0.0))
                dve.wait_ge(s_pe, P_QMM_END)
                inc(nc.vector.tensor_copy(q_sb[0:C, :], qpsA[0:C, :]))
                inc(nc.vector.tensor_copy(q_sb[64:64 + C, :], qpsB[64:64 + C, :]))
                assert t == T_QCOPY
                dve.wait_ge(s_w, 5 * 16)
                dve.wait_ge(s_dve, T_QCOPY)
                for bl in range(HB):
                    for rb in (0, 64):
                        sc = 2 * bl + (1 if rb else 0)
                        inc(nc.vector.scalar_tensor_tensor(
                            out=qb_scr[rb:rb + C, sc * C:(sc + 1) * C],
                            in0=q_sb[rb:rb + C, bl * C:(bl + 1) * C],
                            scalar=1.0,
                            in1=bsrt[rb:rb + C, :],
                            op0=mybir.AluOpType.mult,
                            op1=mybir.AluOpType.mult,
                            accum_out=qb_t[rb:rb + C, bl:bl + 1],
                        ))
                for b in range(B_LOC):
                    bl = b % HB
                    dve.wait_ge(s_pe, P_TP(b))
                    dst = qT_all if b < HB else qTB
                    inc(nc.vector.tensor_copy(dst[0:C, bl * C:(bl + 1) * C],
                                              tp[b % 2][0:C, 0:C]))
                    assert t == T_QT(b)
                dve.wait_ge(s_pe, P_MG2)
                inc(nc.vector.tensor_copy(gtv_r[0:C, :, 0:C], gt_r[0:C, :, 0:C]))
                inc(nc.vector.tensor_copy(gtv_r[64:64 + C, :, 0:C],
                                          gt_r[64:64 + C, :, 0:C]))
                assert t == T_GTV
                for i in range(HB * NCH):
                    pa, chk = divmod(i, NCH)
                    dve.wait_ge(s_pe, P_AT(i))
                    inc(nc.vector.tensor_reduce(
                        out=maxacc[:, i:i + 1],
                        in_=at[i % NAT][:],
                        axis=mybir.AxisListType.X,
                        op=mybir.AluOpType.max,
                    ))
                    assert t == T_TTR(i)
                dve.wait_ge(s_dve, T_TTR(HB * NCH - 1))
                inc(nc.vector.tensor_reduce(
                    out=am1[:],
                    in_=maxacc[:].rearrange("p (b c) -> p b c", b=HB),
                    axis=mybir.AxisListType.X,
                    op=mybir.AluOpType.max,
                ))
                dve.wait_ge(s_dve, T_FIN - 1)
                inc(nc.vector.tensor_add(am_f[:], am1[:], qb_t[:]))
                assert t == T_FIN

    return nc


# ---------------------------------------------------------------- phase 2 ---
def build_phase2() -> bass.Bass:
    nc = bass.Bass()
    # cols 0..MSH-1: WprojT_perm shard; cols MSH..MSH+B-1: vT (both f32r)
    wv = nc.declare_dram_parameter("wv", [N, MSH + B], F32R, isOutput=False)
    am = nc.declare_dram_parameter("am", [B, C], F32, isOutput=False)
    o_sh = nc.declare_dram_parameter("o_sh", [B, MSH, C], F32, isOutput=True)
    W = MSH + B  # 520

    with ExitStack() as ctx:
        ec = ctx.enter_context
        wv_sb = ec(nc.sbuf_tensor("wv_sb", [128, NKC * W], F32R))
        am_sb = ec(nc.sbuf_tensor("am_sb", [128, C], F32))
        pv_sb = ec(nc.sbuf_tensor("pv_sb", [128, MSH], F32))
        ot = [ec(nc.sbuf_tensor(f"ot{i}", [128, MB * C], F32)) for i in range(NMB)]
        pvps = ec(nc.psum_tensor("pvps", [128, MSH], F32))

        s_am2 = ec(nc.semaphore("s_am2"))
        s_in = [ec(nc.semaphore(f"s_in{i}")) for i in range(4)]
        s_pe = ec(nc.semaphore("s_pe"))
        s_dve = ec(nc.semaphore("s_dve"))
        s_out = ec(nc.semaphore("s_out"))

        with nc.Block() as block:

            @block.sync
            def _(sp):
                sp.dma_start(out=am_sb[:], in_=am[:]).then_inc(s_am2, 16)
                for k in range(NKC):
                    sp.dma_start(out=wv_sb[0:KP, k * W:(k + 1) * W],
                                 in_=wv[k * KP:(k + 1) * KP, :]).then_inc(s_in[k // 7], 16)
                for mc in range(NMB):
                    sp.wait_ge(s_dve, 2 + mc)
                    sp.dma_start(
                        out=o_sh[:, mc * MB:(mc + 1) * MB, :], in_=ot[mc][:],
                    ).then_inc(s_out, 16)
                sp.wait_ge(s_out, 16 * NMB)

            @block.tensor
            def _(pe):
                for k in range(NKC):
                    if k % 7 == 0:
                        pe.wait_ge(s_in[k // 7], 16 * 7)
                    mm = nc.tensor.matmul(
                        pvps[:],
                        wv_sb[0:KP, k * W + MSH:k * W + MSH + B],
                        wv_sb[0:KP, k * W:k * W + MSH],
                        start=(k == 0), stop=(k == NKC - 1),
                    )
                    if k == NKC - 1:
                        mm.then_inc(s_pe, 1)

            @block.vector
            def _(dve):
                dve.wait_ge(s_pe, 1)
                nc.vector.tensor_copy(pv_sb[:], pvps[:]).then_inc(s_dve, 1)
                dve.wait_ge(s_am2, 16)  # am present
                dve.wait_ge(s_dve, 1)    # pv copy drained
                for mc in range(NMB):
                    in0 = pv_sb[:, mc * MB:(mc + 1) * MB].broadcast_to((128, MB, C))
                    in1 = am_sb[:, 0:C].unsqueeze(1).broadcast_to((128, MB, C))
                    nc.vector.tensor_tensor(
                        out=ot[mc][:].rearrange("p (m c) -> p m c", m=MB),
                        in0=in0, in1=in1, op=mybir.AluOpType.mult,
                    ).then_inc(s_dve, 1)

    return nc


# ------------------------------------------------------------------- host ---
_CACHE: dict = {}


def _get_programs():
    if "p1" not in _CACHE:
        _CACHE["p1"] = build_phase1()
        _CACHE["p2"] = build_phase2()
    return _CACHE["p1"], _CACHE["p2"]


def _prep_phase1(x, Wq, Wsr, bsr_np):
    import ml_dtypes
    bf = ml_dtypes.bfloat16
    xp = x.reshape(B, C, 7, 8, 7, 8).transpose(0, 1, 3, 5, 2, 4).reshape(B, C, N)
    xp = np.ascontiguousarray(xp).astype(bf)          # (B, C, N) permuted-n
    # per-core DMA-ready layout: [half*NXP + piece, ci, (b_local, XP)]
    xs_h = np.empty((NCORES, 2 * NXP, C, HB * XP), bf)
    for cidx in range(NCORES):
        xc = xp[cidx * B_LOC:(cidx + 1) * B_LOC]      # (16, C, N)
        for h in range(2):
            for j in range(NXP):
                blk = xc[h * HB:(h + 1) * HB, :, j * XP:(j + 1) * XP]  # (8, C, XP)
                xs_h[cidx, h * NXP + j] = blk.transpose(1, 0, 2).reshape(C, HB * XP)
    wq_h = np.ascontiguousarray(
        Wq.transpose(2, 3, 1, 0).reshape(G64, C, C).transpose(1, 0, 2)
    ).reshape(C, G64 * C).astype(bf)                  # [ci, (g, co)]
    wsr_h = np.zeros((128, C), np.float32)
    wsr_h[0:C] = Wsr[:, :, 0, 0]
    wsr_h[64:64 + C] = Wsr[:, :, 0, 0]
    wsr_h = wsr_h.astype(bf)
    ident_h = np.zeros((128, C), np.float32)
    ident_h[0:C] = np.eye(C, dtype=np.float32)
    ident_h[64:64 + C] = np.eye(C, dtype=np.float32)
    bsr_h = np.tile(bsr_np.reshape(1, C), (128, 1)).astype(np.float32)
    return xs_h, wq_h, wsr_h, ident_h, bsr_h


def kernel(x, Wq, Wsr, bsr, Wproj, H, W):
    x = np.asarray(x, np.float32)
    Wq = np.asarray(Wq, np.float32)
    Wsr = np.asarray(Wsr, np.float32)
    bsr_np = np.asarray(bsr, np.float32)
    Wproj = np.asarray(Wproj, np.float32)

    p1, p2 = _get_programs()
    xs_h, wq_h, wsr_h, ident_h, bsr_h = _prep_phase1(x, Wq, Wsr, bsr_np)

    in_maps1 = [{
        "xs": xs_h[c],
        "wq": wq_h, "wsr": wsr_h, "bsr": bsr_h, "ident": ident_h,
    } for c in range(NCORES)]
    res1 = run_bass_kernel_spmd(p1, in_maps1, list(range(NCORES)))

    v_all = np.concatenate([res1.results[c]["v_out"] for c in range(NCORES)], 0)
    am_all = np.empty((B, C), np.float32)
    for c in range(NCORES):
        am_core = res1.results[c]["am_out"]           # (128, 8): [co|64+co, pass]
        am_all[c * B_LOC:c * B_LOC + HB] = am_core[0:C, :].T
        am_all[c * B_LOC + HB:(c + 1) * B_LOC] = am_core[64:64 + C, :].T

    perm = np.arange(N).reshape(7, 8, 7, 8).transpose(1, 3, 0, 2).reshape(N)
    wtp = round_f32r(Wproj.T[perm, :])                              # (n', m)
    vt_h = round_f32r(v_all.T * np.float32(1.0 / 49.0))             # (n', b)
    in_maps2 = []
    for c in range(NCORES):
        wv = np.empty((N, MSH + B), np.float32)
        wv[:, 0:MSH] = wtp[:, c * MSH:(c + 1) * MSH]
        wv[:, MSH:] = vt_h
        in_maps2.append({"wv": wv, "am": am_all})
    res2 = run_bass_kernel_spmd(p2, in_maps2, list(range(NCORES)))

    out = np.concatenate([res2.results[c]["o_sh"] for c in range(NCORES)], 1)
    return out

